# revision 97
# baseline (speedup 1.0000x reference)
"""Trainium2 Bass kernel for nn_DCConv3dKernelPolynomials.

out[o,i,x,n] = sum_b basis_b(position[x,n]) * coeffs[o,i,b]

Strategy (per the sharding hint): shard the 110592 grid points across the 8
NeuronCores (13824 each), replicate the folded coefficient matrix.  The host
re-encodes each point as [sin t, cos t, sin p, cos p, r].  Per core:
 - evaluate the 30 hydrogen-wavefunction basis functions point-major with
   slot-grouped WIDE ops: the 30 products collapse into 6 tensor_tensor
   instructions (radial factor stride-0-broadcast over packed angular
   tiles) plus 4 singles, cutting per-op fixed costs ~2x vs one-op-per-
   basis.  Radial/scalar chains on DVE, tensor-only pack ops on Pool
   (Pool has no PSUM access and no scalar-ptr opcodes on real hw),
   exps on ACT (bf16 outputs),
 - PE-transpose psi with a bf16 identity (1 cyc/row vs 2 for f32), 4
   point-groups packed into the 128 partitions,
 - row-tiled bf16 matmuls (K=30 per 32-row group) against replicated bf16
   coefficients -> PSUM f32, evacuated to SBUF bf16 by ACT during the psi
   phase and ACT+DVE after (psiT copies always DVE: on ACT they stall
   PE's matmuls behind the evac queue), streamed out in 2-4 round chunks.
Latency shaping: pos input DMA split so the first 4 rounds land ~3us;
dummy PE transposes at ~0.4us and ~3.2us start/hold the p-state ramp so
real matmuls run at 2.4GHz; a dummy Exp pulls the 1.3us activation-table
load into the input-DMA window; psi segments [4,6,8,9] rounds ordered so
each engine's in-order stream is ready in sequence.  First output DMA
fires ~8.9us, then the stream runs near the 360GB/s DMA roofline
(20.8us busy).  f16 positions halve input DMA bytes; exp(-r) is derived
as exp(-r/2)^2 on DVE to keep ACT free for PSUM evacuations (the
mid-stream pacer).  Cost model: 35812ns vs 39746ns baseline (seg0 spans 6 rounds so three chunks stream from pos_a's psi; seg1 runs entirely on DVE: cross-engine pair hops cost ~200ns each).
"""
import math

import numpy as np

OUTC, INC = 16, 16
OUTN, CONV_N = 4096, 27
NB = 30
NCORES = 8
PTS = OUTN * CONV_N            # 110592
CPTS = PTS // NCORES           # 13824 per core
NGRP = 4                       # point groups (matmul row tiling)
GPTS = CPTS // NGRP            # 3456 per group
NROUND = GPTS // 128           # 27 transpose rounds
F = NROUND * NGRP              # 108 g-columns (g = 4*c + j)
PI = math.pi

SEG_G = [(0, 24), (24, 48), (48, 80), (80, F)]   # psi segments (g-cols)
CHUNK_ROUNDS = [2, 2, 2, 4, 2, 4, 4, 4, 3]       # output chunks (rounds)
SEG0_G = 24                                      # pos_a covers g-cols [0,24)


# ----------------------------------------------------------------- constants
def _qnums():
    lst = []
    for n in range(1, 5):
        for l in range(0, min(n, 4)):
            for m in range(-l, l + 1):
                if abs(m) <= 3:
                    lst.append((n, l, m))
    return lst


QNUMS = _qnums()


def _laguerre_coeffs(k, alpha):
    return [((-1.0) ** i) * math.comb(k + alpha, k - i) / math.factorial(i)
            for i in range(k + 1)]


def _radial_info(n, l):
    k = n - l - 1
    lag = _laguerre_coeffs(k, 2 * l + 1)
    cr = [lag[i] * (2.0 / n) ** i for i in range(k + 1)]
    norm_r = math.sqrt((2.0 / n) ** 3 * math.factorial(n - l - 1)
                       / (2.0 * n * math.factorial(n + l)))
    lead = cr[-1]
    K_rad = norm_r * (2.0 / n) ** l * lead
    roots = [] if k == 0 else sorted(float(x) for x in
                                     np.real(np.roots(np.array(cr[::-1]))))
    return roots, K_rad


_K_ANG = {(0, 0): 1.0, (1, 0): 1.0, (1, 1): -1.0,
          (2, 0): 1.5, (2, 1): -3.0, (2, 2): 3.0,
          (3, 0): 2.5, (3, 1): -7.5, (3, 2): 15.0, (3, 3): -15.0}
_TRIGFOLD = {0: 1.0, 1: 1.0, -1: 1.0, 2: 2.0, -2: 2.0, 3: 4.0, -3: 4.0}

ROOTS20 = _radial_info(2, 0)[0]
ROOTS30 = _radial_info(3, 0)[0]
ROOTS31 = _radial_info(3, 1)[0]
ROOTS40 = _radial_info(4, 0)[0]
ROOTS41 = _radial_info(4, 1)[0]
ROOTS42 = _radial_info(4, 2)[0]


def _fold_constants():
    K = np.zeros(NB)
    for b, (n, l, m) in enumerate(QNUMS):
        am = abs(m)
        _, K_rad = _radial_info(n, l)
        klm = math.sqrt((2.0 * l + 1.0) / (4.0 * PI)
                        * math.factorial(l - am) / math.factorial(l + am))
        K[b] = (K_rad * klm * (math.sqrt(2.0) if m != 0 else 1.0)
                * _K_ANG[(l, am)] * _TRIGFOLD[m])
    return K


# ------------------------------------------------------------- device program
_PROGRAM_CACHE = {}


def _build_program():
    import concourse.bacc as bacc
    import concourse.tile as tile
    from concourse import mybir

    f32 = mybir.dt.float32
    f16 = mybir.dt.float16
    bf16 = mybir.dt.bfloat16
    AF = mybir.ActivationFunctionType

    nc = bacc.Bacc("TRN2", debug=False, num_devices=NCORES)

    pos_a_d = nc.dram_tensor("pos_a", [128, 5, SEG0_G], f16,
                             kind="ExternalInput")
    pos_b_d = nc.dram_tensor("pos_b", [128, 5, F - SEG0_G], f16,
                             kind="ExternalInput")
    wts_d = nc.dram_tensor("wts", [128, 256], bf16, kind="ExternalInput")
    ident_d = nc.dram_tensor("ident", [128, 128], bf16, kind="ExternalInput")
    out_d = nc.dram_tensor("out", [256, CPTS], bf16, kind="ExternalOutput")

    with tile.TileContext(nc) as tc:
        _kernel_body(tc, nc, out_d.ap(), pos_a_d.ap(), pos_b_d.ap(),
                     wts_d.ap(), ident_d.ap(), f32, f16, bf16, AF)
    nc.compile()
    return nc


def _kernel_body(tc, nc, out_ap, pos_a_ap, pos_b_ap, wts_ap, ident_ap,
                 f32, f16, bf16, AF):
    from contextlib import ExitStack
    from concourse import mybir
    Alu = mybir.AluOpType

    ctx = ExitStack()
    with ctx:
        const = ctx.enter_context(tc.tile_pool(name="const", bufs=1))
        feat = ctx.enter_context(tc.tile_pool(name="feat", bufs=1))
        pT = ctx.enter_context(tc.tile_pool(name="pT", bufs=2, space="PSUM"))
        pM = ctx.enter_context(tc.tile_pool(name="pM", bufs=3, space="PSUM"))
        stg = ctx.enter_context(tc.tile_pool(name="stg", bufs=8))

        act = nc.scalar.activation
        V = nc.vector
        G = nc.gpsimd

        # ---- PE warmup: dummy transposes start the p-state ramp clock
        # (full speed is wall-clock +3us from the FIRST matmul); a second,
        # posTa-dependent one keeps the ramp from resetting during the
        # ~3us input-DMA+prep window (idle > ~4us resets the p-state) ----
        warm_in = const.tile([128, 128], bf16)
        V.memset(warm_in[:], 0.0)
        warm2 = const.tile([128, 128], bf16)
        V.memset(warm2[:], 0.0)
        warm_tp = pT.tile([128, 1024], bf16, tag="tp")
        for _ in range(2):
            nc.tensor.transpose(warm_tp[:, 0:128], warm_in[:], warm_in[:])
        # dummy Exp so the 1.3us activation-table load happens during the
        # input-DMA window, not in front of the first real exp
        warm_act = const.tile([128, 8], f32)
        act(warm_act[:], warm_in[:, 0:8], AF.Exp, scale=-1.0)

        # ---- input DMAs ----
        posTa = feat.tile([128, 5, SEG0_G], f16)
        nc.sync.dma_start(posTa[:], pos_a_ap)
        posTb = feat.tile([128, 5, F - SEG0_G], f16)
        nc.sync.dma_start(posTb[:], pos_b_ap)
        wts = const.tile([128, 256], bf16)
        nc.sync.dma_start(wts[:], wts_ap)
        ident = const.tile([128, 128], bf16)
        nc.sync.dma_start(ident[:], ident_ap)

        # ramp keeper: depends on posTa, fires ~3.2us
        V.tensor_copy(warm2[:, 0:SEG0_G], posTa[:, 0, :])
        nc.tensor.transpose(warm_tp[:, 0:128], warm2[:], warm2[:])

        # broadcastable constants so Pool (no scalar-ptr opcodes) can run
        # the subtract-style ops as plain tensor_tensor
        CONSTS = {}
        for cv in (0.5, 0.25, 0.75, 0.2, 1.0 / 3.0, 0.6):
            t = const.tile([128, 1], f32, name=f"c{len(CONSTS)}",
                           tag=f"c{len(CONSTS)}")
            V.memset(t[:], cv)
            CONSTS[cv] = t
        def cb(cv, w):
            return CONSTS[cv][:, 0:1].broadcast_to([128, w])

        def coords(a, b):
            """(sth, u, s1c1pair, r, r_b3, tile) APs for g-cols [a, b)."""
            t = posTa if b <= SEG0_G else posTb
            o = 0 if b <= SEG0_G else SEG0_G
            w = b - a
            sth = t[:, 0, a - o:b - o]
            u = t[:, 1, a - o:b - o]
            s1 = t[:, 2, a - o:b - o]
            c1 = t[:, 3, a - o:b - o]
            r = t[:, 4, a - o:b - o]
            s1c1 = t[:, 2:4, a - o:b - o].rearrange("p a w -> p w a")
            rb = t[:, 4:5, a - o:b - o].rearrange("p a w -> p w a")
            return sth, u, s1, c1, r, s1c1, rb, w

        # packed intermediates, full-width f32 (E is per-segment: sharing
        # one tile across segments serializes readers behind later
        # segments' writers in the dep tracker)
        Es = [feat.tile([128, g1 - g0, 3], f32, name=f"E{i}", tag=f"E{i}")
              for i, (g0, g1) in enumerate(SEG_G)]
        Er = feat.tile([128, F, 3], f32)       # E * r
        R2 = feat.tile([128, F, 2], f32)       # [R32, E4r2] = Er[1:3] * r
        R43 = feat.tile([128, F], f32)
        R31 = feat.tile([128, F], f32)
        R41a = feat.tile([128, F], f32)
        R41 = feat.tile([128, F], f32)
        R42 = feat.tile([128, F], f32)
        t35 = feat.tile([128, F], f32)
        t41 = feat.tile([128, F], f32)
        t42 = feat.tile([128, F], f32)
        c1sq = feat.tile([128, F], f32)
        u2 = feat.tile([128, F], f32)
        stsq = feat.tile([128, F], f32)
        p33 = feat.tile([128, F], f32)
        T2 = feat.tile([128, F, 2], f32)       # [s2t, c2t]
        T3 = feat.tile([128, F, 2], f32)       # [s3t, c3t]
        Ang1 = feat.tile([128, F, 3], f32)     # [A1s, u, A1c]
        Ang2 = feat.tile([128, F, 5], f32)     # [A2s2, A2s1, p20, A2c1, A2c2]
        Ang3 = feat.tile([128, F, 7], f32)     # [A3s3..A3c3]

        # psi point-major bf16, one tile per segment
        PMs = []
        for i, (g0, g1) in enumerate(SEG_G):
            pm = feat.tile([128, g1 - g0, 32], bf16, tag=f"PM{i}")
            V.memset(pm[:, :, NB:32], 0.0)
            PMs.append(pm)

        def bc(ap_slice, w, k):
            """broadcast a [128, w] (as [128,1,w]) coord or a [128,w,1]
            slot slice across k slots -> [128, w, k]"""
            return ap_slice.broadcast_to([128, w, k])

        def seg(si):
            """Emission order is per-engine issue order (engines are
            in-order): within each engine's stream, ops are listed so each
            becomes ready roughly when its predecessor completes."""
            g0, g1 = SEG_G[si]
            q = slice(g0, g1)
            sth, u, s1, c1, r, s1c1, rb, w = coords(g0, g1)
            PM = PMs[si]
            E = Es[si]
            qE = slice(0, w)
            WV = V                     # main chain engine
            WG = V if si == 1 else G   # pair ops (SBUF-only -> Pool ok)
            WT = V                     # scalar ops illegal on Pool
            WW = V                     # wide products stay on DVE

            # ACT: E3 first (feeds t35 soonest), then E4, E2, direct psi
            act(E[:, qE, 1], r, AF.Exp, scale=-1.0 / 3.0)
            act(E[:, qE, 2], r, AF.Exp, scale=-0.25)
            act(E[:, qE, 0], r, AF.Exp, scale=-0.5)
            # exp(-r) = exp(-r/2)^2: a TT instead of a 4th ACT exp
            WV.tensor_tensor(PM[:, :, 0], E[:, qE, 0], E[:, qE, 0], Alu.mult)
            # POOL: pos-only basics
            G.tensor_tensor(c1sq[:, q], c1, c1, Alu.mult)
            G.tensor_tensor(u2[:, q], u, u, Alu.mult)
            G.tensor_tensor(stsq[:, q], sth, sth, Alu.mult)
            G.tensor_tensor(T2[:, q, 0], s1, c1, Alu.mult)
            G.tensor_tensor(p33[:, q], sth, stsq[:, q], Alu.mult)
            # DVE: pos-only
            WV.tensor_tensor(Ang1[:, q, 0:3:2], bc(sth.unsqueeze(2), w, 2),
                            s1c1, Alu.mult)
            WV.tensor_copy(Ang1[:, q, 1], u)
            # DVE: c1sq-dependent trig (Pool finishes c1sq quickly)
            G.tensor_tensor(T2[:, q, 1], c1sq[:, q], cb(0.5, w),
                            Alu.subtract)
            V.scalar_tensor_tensor(T3[:, q, 0], c1sq[:, q], 0.25, s1,
                                   Alu.subtract, Alu.mult)
            V.scalar_tensor_tensor(T3[:, q, 1], c1sq[:, q], 0.75, c1,
                                   Alu.subtract, Alu.mult)
            # DVE: radial ladder as exps land (E3, E4, then E2)
            WV.scalar_tensor_tensor(t35[:, q], r, ROOTS30[0], E[:, qE, 1],
                                   Alu.subtract, Alu.mult)
            WV.scalar_tensor_tensor(t41[:, q], r, ROOTS40[0], E[:, qE, 2],
                                   Alu.subtract, Alu.mult)
            WV.scalar_tensor_tensor(t42[:, q], r, ROOTS40[1], t41[:, q],
                                   Alu.subtract, Alu.mult)
            WV.tensor_tensor(Er[:, q, :], E[:, qE, :], bc(rb, w, 3), Alu.mult)
            WV.tensor_tensor(R2[:, q, :], Er[:, q, 1:3], bc(rb, w, 2),
                            Alu.mult)
            WV.scalar_tensor_tensor(R31[:, q], r, ROOTS31[0], Er[:, q, 1],
                                   Alu.subtract, Alu.mult)
            WV.scalar_tensor_tensor(R41a[:, q], r, ROOTS41[0], Er[:, q, 2],
                                   Alu.subtract, Alu.mult)
            WV.scalar_tensor_tensor(R41[:, q], r, ROOTS41[1], R41a[:, q],
                                   Alu.subtract, Alu.mult)
            WV.scalar_tensor_tensor(R42[:, q], r, ROOTS42[0], R2[:, q, 1],
                                   Alu.subtract, Alu.mult)
            # R43 tail + later the 7-wide product
            WV.tensor_tensor(R43[:, q], R2[:, q, 1], r, Alu.mult)
            # DVE: psi singles
            WV.scalar_tensor_tensor(PM[:, :, 1], r, ROOTS20[0], E[:, qE, 0],
                                   Alu.subtract, Alu.mult)
            WV.scalar_tensor_tensor(PM[:, :, 5], r, ROOTS30[1], t35[:, q],
                                   Alu.subtract, Alu.mult)
            WV.scalar_tensor_tensor(PM[:, :, 14], r, ROOTS40[2], t42[:, q],
                                   Alu.subtract, Alu.mult)
            # angular ladders
            G.tensor_tensor(Ang2[:, q, 2], u2[:, q], cb(1.0 / 3.0, w),
                            Alu.subtract)
            WG.tensor_tensor(Ang2[:, q, 1:4:2], bc(u.unsqueeze(2), w, 2),
                             Ang1[:, q, 0:3:2], Alu.mult)
            WG.tensor_tensor(Ang2[:, q, 0:5:4], bc(stsq[:, q].unsqueeze(2),
                                                   w, 2),
                             T2[:, q, :], Alu.mult)
            V.scalar_tensor_tensor(Ang3[:, q, 2:5:2], bc(
                u2[:, q].unsqueeze(2), w, 2), 0.2, Ang1[:, q, 0:3:2],
                Alu.subtract, Alu.mult)
            V.scalar_tensor_tensor(Ang3[:, q, 3], u2[:, q], 0.6, u,
                                   Alu.subtract, Alu.mult)
            WG.tensor_tensor(Ang3[:, q, 1:6:4], bc(u.unsqueeze(2), w, 2),
                             Ang2[:, q, 0:5:4], Alu.mult)
            WG.tensor_tensor(Ang3[:, q, 0:7:6], bc(p33[:, q].unsqueeze(2),
                                                   w, 2),
                             T3[:, q, :], Alu.mult)
            # wide products: PM[s0:s1] = radial (bcast) * AngK
            WW.tensor_tensor(PM[:, :, 2:5], bc(Er[:, q, 0:1], w, 3),
                            Ang1[:, q, :], Alu.mult)
            WW.tensor_tensor(PM[:, :, 6:9], bc(R31[:, q].unsqueeze(2), w, 3),
                            Ang1[:, q, :], Alu.mult)
            WW.tensor_tensor(PM[:, :, 9:14], bc(R2[:, q, 0:1], w, 5),
                            Ang2[:, q, :], Alu.mult)
            WW.tensor_tensor(PM[:, :, 15:18], bc(R41[:, q].unsqueeze(2), w, 3),
                            Ang1[:, q, :], Alu.mult)
            WW.tensor_tensor(PM[:, :, 18:23], bc(R42[:, q].unsqueeze(2), w, 5),
                            Ang2[:, q, :], Alu.mult)
            WW.tensor_tensor(PM[:, :, 23:30], bc(R43[:, q].unsqueeze(2), w, 7),
                             Ang3[:, q, :], Alu.mult)

        # ---- transpose / matmul / evac / DMA stream ----
        psiT = feat.tile([128, GPTS], bf16)
        out3 = out_ap.rearrange("o (j p) -> o j p", j=NGRP)

        def pm_for_round(r):
            for (g0, g1), pm in zip(SEG_G, PMs):
                if g0 <= 4 * r < g1:
                    return pm, g0
            raise AssertionError

        # GPSIMD cannot touch PSUM on real hw: evacs are DVE/ACT only
        EVAC_LATE = [nc.scalar.copy, V.tensor_copy]
        EVAC_MID = [nc.scalar.copy]
        EVAC_FILL = [nc.scalar.copy]
        evac_i = [0]

        def chunk(ci, r0, nr, hs=(0, 1), evl=None):
            t0 = r0 * 128
            n = nr * 128
            if 0 in hs:
                tp = pT.tile([128, 1024], bf16, tag="tp")
                for ri in range(nr):
                    r = r0 + ri
                    pm, g0 = pm_for_round(r)
                    nc.tensor.transpose(tp[:, ri * 128:(ri + 1) * 128],
                                        pm[:, 4 * r - g0:4 * r - g0 + 4, :],
                                        ident[:])
                V.tensor_copy(psiT[:, t0:t0 + n], tp[:, :n])
            for h in hs:
                def mm(j, ps, col0):
                    lhsT = wts[32 * j:32 * j + NB, 128 * h:128 * (h + 1)]
                    rhs = psiT[32 * j:32 * j + NB, t0:t0 + n]
                    nc.tensor.matmul(ps[:, col0:col0 + n], lhsT, rhs,
                                     start=True, stop=True,
                                     tile_position=(32 * j, 0))

                def ev_op():
                    lst = evl if evl is not None else (
                        EVAC_FILL if ci <= 1 else
                        EVAC_MID if ci <= 3 else EVAC_LATE)
                    e = lst[evac_i[0] % len(lst)]
                    evac_i[0] += 1
                    return e

                so = stg.tile([128, 2048], bf16, tag="so")
                so4 = so.rearrange("p (j q) -> p j q", q=512)
                if n == 256:
                    # one PSUM tile per j-pair, j in separate banks (two
                    # start=True matmuls must not share a PSUM bank)
                    for jp in (0, 2):
                        ps = pM.tile([128, 1024], f32, tag="ps")
                        ps4 = ps.rearrange("p (a q) -> p a q", q=256)
                        mm(jp, ps, 0)
                        mm(jp + 1, ps, 512)
                        ev_op()(so4[:, jp:jp + 2, :n], ps4[:, 0:3:2, :])
                        if ci == 0:
                            dst = out3[128 * h:128 * (h + 1), jp:jp + 2,
                                       t0:t0 + n]
                            nc.sync.dma_start(dst, so4[:, jp:jp + 2, :n])
                    if ci != 0:
                        dst = out3[128 * h:128 * (h + 1), :, t0:t0 + n]
                        nc.sync.dma_start(dst, so4[:, :, :n])
                else:
                    for jp in (0, 2):
                        ps = pM.tile([128, 1024], f32, tag="ps")
                        mm(jp, ps, 0)
                        mm(jp + 1, ps, 512)
                        if n == 512:
                            ev_op()(so[:, jp * 512:jp * 512 + 1024],
                                    ps[:, :1024])
                        else:
                            ev_op()(so[:, jp * 512:jp * 512 + n], ps[:, :n])
                            ev_op()(so[:, (jp + 1) * 512:(jp + 1) * 512 + n],
                                    ps[:, 512:512 + n])
                    dst = out3[128 * h:128 * (h + 1), :, t0:t0 + n]
                    nc.sync.dma_start(dst, so4[:, :, :n])

        # emission order = scheduler priority
        r0s = []
        acc = 0
        for nr in CHUNK_ROUNDS:
            r0s.append(acc)
            acc += nr

        seg(0)
        # second ramp keeper (~5us, depends on a seg0 wide product)
        V.tensor_copy(warm2[:, 0:8], PMs[0][:, 0:8, 3])
        nc.tensor.transpose(warm_tp[:, 0:128], warm2[:], warm2[:])
        chunk(0, r0s[0], CHUNK_ROUNDS[0])
        chunk(1, r0s[1], CHUNK_ROUNDS[1])
        seg(1)
        chunk(2, r0s[2], CHUNK_ROUNDS[2])
        chunk(3, r0s[3], CHUNK_ROUNDS[3])
        seg(2)
        seg(3)
        for ci in range(4, len(CHUNK_ROUNDS)):
            chunk(ci, r0s[ci], CHUNK_ROUNDS[ci])


def _get_program():
    if "nc" not in _PROGRAM_CACHE:
        _PROGRAM_CACHE["nc"] = _build_program()
    return _PROGRAM_CACHE["nc"]


# ---------------------------------------------------------------- host wrapper
def _host_prep(position, coeffs):
    import ml_dtypes

    K = _fold_constants()
    Cs = (np.asarray(coeffs, np.float64).reshape(OUTC * INC, NB)
          * K[None, :])
    W = np.zeros((128, 256), np.float64)
    for j in range(NGRP):
        W[32 * j:32 * j + NB, :] = Cs.T
    W = W.astype(ml_dtypes.bfloat16)
    pts = np.asarray(position, np.float64).reshape(PTS, 3)
    r, th, ph = pts[:, 0], pts[:, 1], pts[:, 2]
    X = np.stack([np.sin(th), np.cos(th), np.sin(ph), np.cos(ph), r],
                 axis=1).astype(np.float16)
    pos_cores = []
    for k in range(NCORES):
        sl = X[k * CPTS:(k + 1) * CPTS]
        v = sl.reshape(NGRP, NROUND, 128, 5)       # [j, c, p, coord]
        v = np.transpose(v, (2, 3, 1, 0))          # [p, coord, c, j]
        v = np.ascontiguousarray(v.reshape(128, 5, F))
        pos_cores.append((np.ascontiguousarray(v[:, :, :SEG0_G]),
                          np.ascontiguousarray(v[:, :, SEG0_G:])))
    return pos_cores, W


def kernel(position, coeffs, _collect=None):
    import ml_dtypes
    from concourse.bass_utils import run_bass_kernel_spmd

    pos_cores, W = _host_prep(position, coeffs)
    ident = np.eye(128, dtype=ml_dtypes.bfloat16)
    in_maps = [{"pos_a": pos_cores[k][0], "pos_b": pos_cores[k][1],
                "wts": W, "ident": ident}
               for k in range(NCORES)]
    nc = _get_program()
    try:
        res = run_bass_kernel_spmd(nc, in_maps, core_ids=list(range(NCORES)))
    except Exception:
        # transient NRT/axon failures usually clear on retry
        res = run_bass_kernel_spmd(nc, in_maps, core_ids=list(range(NCORES)))
    if _collect is not None:
        _collect.append(res)
    full = np.concatenate(
        [np.asarray(res.results[k]["out"]).astype(np.float32)
         for k in range(NCORES)], axis=1)
    return full.reshape(OUTC, INC, OUTN, CONV_N)
